# revision 11
# baseline (speedup 1.0000x reference)
"""Multi-head self-attention (B=2, S=4096, D=512, H=8, Dh=64) on 8 TRN2 cores.

Sharding: core i handles batch b = i//4 and head-pair hp = i%4 (heads 2*hp,
2*hp+1).  Each core computes Q/K/V projections for its two heads, flash-style
attention (no-max softmax; scores range is +-9 so exp is safe), and a partial
out-projection.  Host sums the 4 partial outputs per batch and transposes back.

v2 design (PE/ACT/DVE co-balanced, all-fp16 datapath):
  - All matmul operands fp16 (10-bit mantissa ~ f32r accuracy class).  fp16
    stationary operands get hidden LDWEIGHTS (pull-ahead) + fast weight load.
  - Scores for the two heads are emitted back-to-back with base_partition
    0/64 slices -> tile_position (0,0)/(64,0) row groups -> the PE runs them
    CONCURRENTLY (row tiling), halving scores stream time.
  - exp() alternates between ACT (exact, activation Exp) and DVE (Schraudolph
    int16 bit trick: e^(s/8) bits ~= trunc(1024*log2e*s/8 + C); the 1024*
    log2e/8 factor is folded into Wq host-side so scores arrive pre-scaled).
    One fused [128, 1024] tile covers both heads per k-tile.
  - V with a ones column appended per k-tile ([128, 65] weights); matmul with
    it accumulates context AND the softmax denominator (row 64) in one pass.
  - Normalize: ctx psum -> SBUF via DMA, reciprocal_approx_fast on the sums
    row, 1/sigma broadcast via DRAM stride-0 bounce, one DVE mul per head.
  - Out-projection per q-block; outputs DMA'd straight from PSUM to DRAM.

TRN2 quirk: walrus encodes only ONE sync wait on TPB compute instructions.
`_legalize_matmul_waits` moves extra waits onto injected single-wait no-ops.
"""

import sys
from contextlib import ExitStack

for _p in ("/opt/trn_rl_repo",):
    if _p not in sys.path:
        sys.path.insert(0, _p)

import numpy as np

import concourse.bass as bass
import concourse.tile as tile
from concourse import mybir
from concourse.bass_utils import run_bass_kernel_spmd

F32 = mybir.dt.float32
F16 = mybir.dt.float16
I16 = mybir.dt.int16
D = 512          # model dim
DH = 64          # head dim
P = 128          # partitions
B = 2
H = 8
S_FULL = 4096
N_CORES = 8
NC_T = D // P    # 4 contraction tiles over model dim

# Schraudolph fp16 exp: bits(e^(s/8)) ~= trunc(s_scaled + SCHRAU_C) where
# s_scaled = (1024*log2e/8) * s arrives pre-scaled (folded into Wq).
LAM16 = 1024.0 * np.log2(np.e) / 8.0          # 184.6644...
SCHRAU_C = 15315.75                            # tuned: max rel err +-3.0e-2
ACT_SCALE = float(np.log(2.0) / 1024.0)        # exp(ACT_SCALE * s_scaled)
# k-tiles whose exp runs on DVE (Schraudolph) instead of ACT (exact)
DVE_KMOD = (2, 5, 7)                           # 3 of 8 -> 37.5% on DVE

LAST_RESULTS = None  # test harness reads exec_time_ns from here


def _emit(nc: bass.Bass, tc: "tile.TileContext", ctx: ExitStack, S: int):
    NS = S // 512            # 512-wide seq blocks
    NK = S // P              # 128-row key tiles
    NQB = S // 512           # q blocks

    def mm(out, lhsT, rhs, start=True, stop=True):
        return nc.tensor.matmul(out, lhsT, rhs, start=start, stop=stop)

    xt = nc.declare_dram_parameter("xt", [D, S], F16, isOutput=False)
    wqkv = nc.declare_dram_parameter("wqkv", [D, 3 * P], F16, isOutput=False)
    wo = nc.declare_dram_parameter("wo", [P, D], F16, isOutput=False)
    yt = nc.declare_dram_parameter("yt", [D, S], F32, isOutput=True)

    const = ctx.enter_context(tc.tile_pool(name="const", bufs=1))

    # ---- weights to SBUF (one merged DMA per chunk + wo) ----
    wqkv_sb = []
    for c in range(NC_T):
        t = const.tile([P, 3 * P], F16, tag=f"wqkv{c}", name=f"wqkv{c}")
        nc.sync.dma_start(out=t[:], in_=wqkv[c * P:(c + 1) * P, :])
        wqkv_sb.append(t)
    w_sb = {name: [t[:, i * P:(i + 1) * P] for t in wqkv_sb]
            for i, name in enumerate(("wq", "wk", "wv"))}
    wo_sb = const.tile([P, D], F16, tag="wo")
    nc.sync.dma_start(out=wo_sb[:], in_=wo[:, :])

    # ---- xt to SBUF, one tile per (chunk, 1024-block), j-major order so
    # compute pipelines with the load; split across the two HWDGE queues ----
    NB = max(1, S // 1024)
    BW = min(S, 1024)
    xtc = [[None] * NB for _ in range(NC_T)]
    qeng = [nc.sync, nc.scalar]
    for j in range(NB):
        for c in range(NC_T):
            t = const.tile([P, BW], F16, tag=f"xt{c}_{j}", name=f"xt{c}_{j}")
            qeng[(j * NC_T + c) % 2].dma_start(
                out=t[:], in_=xt[c * P:(c + 1) * P, j * BW:(j + 1) * BW])
            xtc[c][j] = t

    def xts(c, j512):
        """[128, 512] view of xt chunk c, 512-block j512."""
        t = xtc[c][j512 // (BW // 512)]
        off = (j512 % (BW // 512)) * 512
        return t[:, off:off + 512]

    # persistent intermediates (all fp16)
    qt_sb = const.tile([P, S], F16, tag="qt")      # [2*64 d, S], pre-scaled
    kt_sb = const.tile([P, S], F16, tag="kt")
    # V padded to 128 cols per k-tile: [v(64) | ones(1) | zeros(63)].  The
    # 128-wide fp16 weight gets fast-weight-load; rows 65:128 of the ctx psum
    # accumulate zeros and are ignored.
    VW = P
    vones = [const.tile([P, NK * VW], F16, tag=f"vones{h}",
                        name=f"vones{h}") for h in range(2)]
    for h in range(2):
        vv = vones[h].rearrange("p (k c) -> p k c", c=VW)
        nc.vector.memset(vv[:, :, DH:], 0.0)
        nc.vector.memset(vv[:, :, DH:DH + 1], 1.0)

    # ---- phase A: projections ----
    with tc.tile_pool(name="pa", bufs=2, space="PSUM") as pa:
        for j in range(NS):
            jsl = slice(j * 512, (j + 1) * 512)
            pq = pa.tile([P, 512], F32, tag="pq", name="pq")
            for c in range(NC_T):
                mm(pq[:], w_sb["wq"][c], xts(c, j),
                   start=(c == 0), stop=(c == NC_T - 1))
            nc.vector.tensor_copy(qt_sb[:, jsl], pq[:])
            pk = pa.tile([P, 512], F32, tag="pk", name="pk")
            for c in range(NC_T):
                mm(pk[:], w_sb["wk"][c], xts(c, j),
                   start=(c == 0), stop=(c == NC_T - 1))
            nc.scalar.copy(kt_sb[:, jsl], pk[:])
            for t in range(4):
                k = j * 4 + t
                tsl = slice(t * P, (t + 1) * P)
                pv = pa.tile([P, P], F32, tag="pv", name="pv")
                for c in range(NC_T):
                    mm(pv[:], xts(c, j)[:, tsl], w_sb["wv"][c],
                       start=(c == 0), stop=(c == NC_T - 1))
                nc.vector.tensor_copy(
                    vones[0][:, k * VW:k * VW + DH], pv[:, 0:DH])
                nc.scalar.copy(
                    vones[1][:, k * VW:k * VW + DH], pv[:, DH:P])

    # ---- phase B: attention + phase C fused per q-block ----
    # PSUM budget: s 3x[128,1024]=6 banks + ctxo 1x[128,1024]=2 banks = 8.
    # The ctxo ring serves ctx2(qb) -> o0(qb) -> o1(qb) -> ctx2(qb+1); the
    # resulting early-qb ctx-MM stall is absorbed by the deep e-tile ring.
    ps = ctx.enter_context(tc.tile_pool(name="ps", bufs=3, space="PSUM"))
    es = ctx.enter_context(tc.tile_pool(name="es", bufs=10))
    cu = ctx.enter_context(tc.tile_pool(name="cu", bufs=2))
    sv = ctx.enter_context(tc.tile_pool(name="sv", bufs=2))
    bcp = ctx.enter_context(tc.tile_pool(name="bcp", bufs=2))
    csp = ctx.enter_context(tc.tile_pool(name="csp", bufs=2))
    osp = ctx.enter_context(tc.tile_pool(name="osp", bufs=2))
    rdp = ctx.enter_context(tc.tile_pool(name="rdp", bufs=2, space="DRAM"))

    for qb in range(NQB):
        qsl = slice(qb * 512, (qb + 1) * 512)
        # ctx accumulator: h0 in cols 0:512, h1 in cols 512:1024; row 64 =
        # sums; rows 65:128 accumulate zeros (padded V weights)
        ctx2 = ps.tile([P, 1024], F32, tag="ctxo", bufs=1, name="ctx2")
        for k in range(NK):
            ksl = slice(k * P, (k + 1) * P)
            sp = ps.tile([P, 1024], F32, tag="s", name="sp")
            # two heads' scores: base_partition 0/64 slices -> row-tiled pair
            for h in range(2):
                hsl = slice(h * DH, (h + 1) * DH)
                mm(sp[:, h * 512:(h + 1) * 512],
                   kt_sb[hsl, ksl], qt_sb[hsl, qsl])
            e = es.tile([P, 1024], I16, tag="e", name="e")
            if (k % 8) in DVE_KMOD:
                # Schraudolph: int16 bits of fp16 e^(s/8)
                nc.vector.tensor_scalar(e[:], sp[:], SCHRAU_C, None,
                                        mybir.AluOpType.add)
            else:
                nc.scalar.activation(e[:].bitcast(F16), sp[:],
                                     mybir.ActivationFunctionType.Exp,
                                     scale=ACT_SCALE)
            ef = e[:].bitcast(F16)
            for h in range(2):
                vo = vones[h][:, k * VW:(k + 1) * VW]
                mm(ctx2[:, h * 512:(h + 1) * 512], vo,
                   ef[:, h * 512:(h + 1) * 512],
                   start=(k == 0), stop=(k == NK - 1))
        # drain ctx (unnormalized) to SBUF, freeing the psum bank (ACT copy)
        ctxU = cu.tile([DH + 1, 1024], F32, tag="cu", name="ctxU")
        nc.scalar.copy(ctxU[:], ctx2[0:DH + 1, :])
        # 1/sigma: DMA-reshape the sums row to [64, 16] (DVE reciprocal is
        # 8 cyc/elem along free dim -- spread it across partitions), exact
        # reciprocal, then broadcast via DRAM stride-0 bounce
        sg = sv.tile([DH, 16], F32, tag="sg", name="sg")
        nc.sync.dma_start(out=sg[:], in_=ctxU[DH:DH + 1, :])
        sr = sv.tile([DH, 16], F32, tag="sr", name="sr")
        nc.vector.reciprocal(sr[:], sg[:])
        sd = rdp.tile([1, 1024], F32, tag="sd", name="sd")
        nc.sync.dma_start(out=sd[0:1, :], in_=sr[:])
        bc = bcp.tile([DH, 1024], F32, tag="bc", name="bc")
        sd_bcast = bass.AP(tensor=sd.tensor, offset=sd.offset,
                           ap=[[0, DH]] + list(sd[0:1, :].ap)[1:])
        nc.sync.dma_start(out=bc[:], in_=sd_bcast)
        # normalized ctx, fp16, heads stacked on partitions for out-proj
        # (on GPSIMD: keeps ACT/DVE free for exp; SBUF-only operands)
        ctxs = csp.tile([P, 512], F16, tag="ctxs", name="ctxs")
        for h in range(2):
            nc.gpsimd.tensor_mul(ctxs[h * DH:(h + 1) * DH, :],
                                 ctxU[0:DH, h * 512:(h + 1) * 512],
                                 bc[:, h * 512:(h + 1) * 512])
        # phase C for this q-block: 4 output-chunk matmuls in 2 psum pairs
        # (allocated from the ctxo ring, right after ctx2 drains); drain
        # pairs via alternating ACT/DVE copies, then DMA out
        for pr in range(2):
            o_ps = ps.tile([P, 1024], F32, tag="ctxo", bufs=1, name="o_ps")
            for i in range(2):
                e4 = pr * 2 + i
                mm(o_ps[:, i * 512:(i + 1) * 512],
                   wo_sb[:, e4 * P:(e4 + 1) * P], ctxs[:])
            o_sb = osp.tile([P, 1024], F32, tag="osb", name="o_sb")
            if pr == 0:
                nc.scalar.copy(o_sb[:], o_ps[:])
            else:
                nc.vector.tensor_copy(o_sb[:], o_ps[:])
            for i in range(2):
                e4 = pr * 2 + i
                nc.sync.dma_start(out=yt[e4 * P:(e4 + 1) * P, qsl],
                                  in_=o_sb[:, i * 512:(i + 1) * 512])


_TPB_ENGINES = {mybir.EngineType.PE, mybir.EngineType.Activation,
                mybir.EngineType.DVE, mybir.EngineType.Pool}


def _legalize_matmul_waits(nc: bass.Bass) -> int:
    """Walrus encodes only ONE sync wait on TPB compute instructions (seen on
    Matmult and TensorCopy).  Move extra waits onto injected same-engine
    no-ops (one wait each) placed immediately before the instruction in its
    block: same semantics, legal encoding."""
    n_fixed = 0
    for f in nc.m.functions:
        for bb in f.blocks:
            out = []
            changed = False
            for ins in bb.instructions:
                si = ins.sync_info
                if (getattr(ins, "engine", None) is not None
                        and si is not None and len(si.on_wait) > 1):
                    for idx, w in enumerate(si.on_wait[:-1]):
                        nop = mybir.InstNoOp(name=f"{ins.name}-lgw{idx}",
                                             ins=[], outs=[])
                        nop.engine = ins.engine
                        nop.sync_info = mybir.SyncInfo(on_wait=[w], on_update=[])
                        out.append(nop)
                    ins.sync_info = mybir.SyncInfo(on_wait=[si.on_wait[-1]],
                                                   on_update=si.on_update)
                    n_fixed += 1
                    changed = True
                out.append(ins)
            if changed:
                bb.instructions = out
    return n_fixed


def build(S: int = S_FULL, legalize: bool = False) -> bass.Bass:
    nc = bass.Bass()
    with ExitStack() as ctx:
        ctx.enter_context(nc.allow_low_precision(
            reason="fp16 matmul operands / int16 exp bit-trick"))
        tc = ctx.enter_context(tile.TileContext(nc))
        _emit(nc, tc, ctx, S)
    if legalize:
        # only for the walrus/hardware path; CoreSim wants updates on every
        # instruction and doesn't enforce the 1-wait Matmult limit
        _legalize_matmul_waits(nc)
    return nc


_NC_CACHE = {}


def _get_nc(S: int) -> bass.Bass:
    if S not in _NC_CACHE:
        _NC_CACHE[S] = build(S, legalize=True)
    return _NC_CACHE[S]


def make_in_maps(X, Wq, Wk, Wv, Wo):
    X = np.asarray(X, np.float32)
    Wq = np.asarray(Wq, np.float32)
    Wk = np.asarray(Wk, np.float32)
    Wv = np.asarray(Wv, np.float32)
    Wo = np.asarray(Wo, np.float32)
    xts = [np.ascontiguousarray(X[b].T).astype(np.float16) for b in range(B)]
    in_maps = []
    for i in range(N_CORES):
        b, hp = divmod(i, 4)  # 4 head-pairs per batch
        csl = slice(hp * P, (hp + 1) * P)
        wqkv = np.concatenate([
            # fold the Schraudolph/ACT pre-scale into Wq
            Wq[:, csl] * LAM16, Wk[:, csl], Wv[:, csl]], axis=1)
        in_maps.append({
            "xt": xts[b],
            "wqkv": np.ascontiguousarray(wqkv).astype(np.float16),
            "wo": np.ascontiguousarray(Wo[csl, :]).astype(np.float16),
        })
    return in_maps


def kernel(X, Wq, Wk, Wv, Wo, _trace=False):
    global LAST_RESULTS
    X = np.asarray(X, dtype=np.float32)
    S = X.shape[1]
    nc = _get_nc(S)
    in_maps = make_in_maps(X, Wq, Wk, Wv, Wo)
    res = run_bass_kernel_spmd(nc, in_maps, list(range(N_CORES)), trace=_trace)
    LAST_RESULTS = res
    Y = np.zeros((B, S, D), dtype=np.float32)
    for i in range(N_CORES):
        Y[i // 4] += res.results[i]["yt"].T
    return Y


# revision 14
# speedup vs baseline: 1.0150x; 1.0150x over previous
"""Multi-head self-attention (B=2, S=4096, D=512, H=8, Dh=64) on 8 TRN2 cores.

Sharding: core i handles batch b = i//4 and head-pair hp = i%4 (heads 2*hp,
2*hp+1).  Each core computes Q/K/V projections for its two heads, flash-style
attention (no-max softmax; scores range is +-9 so exp is safe), and a partial
out-projection.  Host sums the 4 partial outputs per batch and transposes back.

v2 design (PE/ACT/DVE co-balanced, all-fp16 datapath):
  - All matmul operands fp16 (10-bit mantissa ~ f32r accuracy class).  fp16
    stationary operands get hidden LDWEIGHTS (pull-ahead) + fast weight load.
  - Scores for the two heads are emitted back-to-back with base_partition
    0/64 slices -> tile_position (0,0)/(64,0) row groups -> the PE runs them
    CONCURRENTLY (row tiling), halving scores stream time.
  - exp() alternates between ACT (exact, activation Exp) and DVE (Schraudolph
    int16 bit trick: e^(s/8) bits ~= trunc(1024*log2e*s/8 + C); the 1024*
    log2e/8 factor is folded into Wq host-side so scores arrive pre-scaled).
    One fused [128, 1024] tile covers both heads per k-tile.
  - V with a ones column appended per k-tile ([128, 65] weights); matmul with
    it accumulates context AND the softmax denominator (row 64) in one pass.
  - Normalize: ctx psum -> SBUF via DMA, reciprocal_approx_fast on the sums
    row, 1/sigma broadcast via DRAM stride-0 bounce, one DVE mul per head.
  - Out-projection per q-block; outputs DMA'd straight from PSUM to DRAM.

TRN2 quirk: walrus encodes only ONE sync wait on TPB compute instructions.
`_legalize_matmul_waits` moves extra waits onto injected single-wait no-ops.
"""

import sys
from contextlib import ExitStack

for _p in ("/opt/trn_rl_repo",):
    if _p not in sys.path:
        sys.path.insert(0, _p)

import numpy as np

import concourse.bass as bass
import concourse.tile as tile
from concourse import mybir
from concourse.bass_utils import run_bass_kernel_spmd

F32 = mybir.dt.float32
F16 = mybir.dt.float16
I16 = mybir.dt.int16
D = 512          # model dim
DH = 64          # head dim
P = 128          # partitions
B = 2
H = 8
S_FULL = 4096
N_CORES = 8
NC_T = D // P    # 4 contraction tiles over model dim

# Schraudolph fp16 exp: bits(e^(s/8)) ~= trunc(s_scaled + SCHRAU_C) where
# s_scaled = (1024*log2e/8) * s arrives pre-scaled (folded into Wq).
LAM16 = 1024.0 * np.log2(np.e) / 8.0          # 184.6644...
SCHRAU_C = 15315.75                            # tuned: max rel err +-3.0e-2
ACT_SCALE = float(np.log(2.0) / 1024.0)        # exp(ACT_SCALE * s_scaled)
# k-tiles whose exp runs on DVE (Schraudolph) instead of ACT (exact)
def _use_dve(k: int) -> bool:
    return (k % 8) in (2, 5, 7) or (k % 32) == 8   # 13/32 -> 40.6% on DVE

LAST_RESULTS = None  # test harness reads exec_time_ns from here


def _emit(nc: bass.Bass, tc: "tile.TileContext", ctx: ExitStack, S: int):
    NS = S // 512            # 512-wide seq blocks
    NK = S // P              # 128-row key tiles
    NQB = S // 512           # q blocks

    def mm(out, lhsT, rhs, start=True, stop=True):
        return nc.tensor.matmul(out, lhsT, rhs, start=start, stop=stop)

    xt = nc.declare_dram_parameter("xt", [D, S], F16, isOutput=False)
    wqkv = nc.declare_dram_parameter("wqkv", [D, 3 * P], F16, isOutput=False)
    wo = nc.declare_dram_parameter("wo", [P, D], F16, isOutput=False)
    yt = nc.declare_dram_parameter("yt", [D, S], F32, isOutput=True)

    const = ctx.enter_context(tc.tile_pool(name="const", bufs=1))

    # ---- weights to SBUF (one merged DMA per chunk + wo) ----
    wqkv_sb = []
    for c in range(NC_T):
        t = const.tile([P, 3 * P], F16, tag=f"wqkv{c}", name=f"wqkv{c}")
        nc.sync.dma_start(out=t[:], in_=wqkv[c * P:(c + 1) * P, :])
        wqkv_sb.append(t)
    w_sb = {name: [t[:, i * P:(i + 1) * P] for t in wqkv_sb]
            for i, name in enumerate(("wq", "wk", "wv"))}
    wo_sb = const.tile([P, D], F16, tag="wo")
    nc.sync.dma_start(out=wo_sb[:], in_=wo[:, :])

    # ---- xt to SBUF, one tile per (chunk, 1024-block), j-major order so
    # compute pipelines with the load; split across the two HWDGE queues ----
    NB = max(1, S // 1024)
    BW = min(S, 1024)
    xtc = [[None] * NB for _ in range(NC_T)]
    qeng = [nc.sync, nc.scalar]
    for j in range(NB):
        for c in range(NC_T):
            t = const.tile([P, BW], F16, tag=f"xt{c}_{j}", name=f"xt{c}_{j}")
            qeng[(j * NC_T + c) % 2].dma_start(
                out=t[:], in_=xt[c * P:(c + 1) * P, j * BW:(j + 1) * BW])
            xtc[c][j] = t

    def xts(c, j512):
        """[128, 512] view of xt chunk c, 512-block j512."""
        t = xtc[c][j512 // (BW // 512)]
        off = (j512 % (BW // 512)) * 512
        return t[:, off:off + 512]

    # persistent intermediates (all fp16)
    qt_sb = const.tile([P, S], F16, tag="qt")      # [2*64 d, S], pre-scaled
    kt_sb = const.tile([P, S], F16, tag="kt")
    # V padded to 128 cols per k-tile: [v(64) | ones(1) | zeros(63)].  The
    # 128-wide fp16 weight gets fast-weight-load; rows 65:128 of the ctx psum
    # accumulate zeros and are ignored.
    VW = P
    vones = [const.tile([P, NK * VW], F16, tag=f"vones{h}",
                        name=f"vones{h}") for h in range(2)]
    for h in range(2):
        vv = vones[h].rearrange("p (k c) -> p k c", c=VW)
        nc.vector.memset(vv[:, :, DH:], 0.0)
        nc.vector.memset(vv[:, :, DH:DH + 1], 1.0)

    # ---- phase A: projections ----
    with tc.tile_pool(name="pa", bufs=2, space="PSUM") as pa:
        for j in range(NS):
            jsl = slice(j * 512, (j + 1) * 512)
            pq = pa.tile([P, 512], F32, tag="pq", name="pq")
            for c in range(NC_T):
                mm(pq[:], w_sb["wq"][c], xts(c, j),
                   start=(c == 0), stop=(c == NC_T - 1))
            nc.vector.tensor_copy(qt_sb[:, jsl], pq[:])
            pk = pa.tile([P, 512], F32, tag="pk", name="pk")
            for c in range(NC_T):
                mm(pk[:], w_sb["wk"][c], xts(c, j),
                   start=(c == 0), stop=(c == NC_T - 1))
            nc.scalar.copy(kt_sb[:, jsl], pk[:])
            for t in range(4):
                k = j * 4 + t
                tsl = slice(t * P, (t + 1) * P)
                pv = pa.tile([P, P], F32, tag="pv", name="pv")
                for c in range(NC_T):
                    mm(pv[:], xts(c, j)[:, tsl], w_sb["wv"][c],
                       start=(c == 0), stop=(c == NC_T - 1))
                nc.vector.tensor_copy(
                    vones[0][:, k * VW:k * VW + DH], pv[:, 0:DH])
                nc.scalar.copy(
                    vones[1][:, k * VW:k * VW + DH], pv[:, DH:P])

    # ---- phase B: attention + phase C fused per q-block ----
    # PSUM budget: s 3x[128,1024]=6 banks + ctxo 1x[128,1024]=2 banks = 8.
    # The ctxo ring serves ctx2(qb) -> o0(qb) -> o1(qb) -> ctx2(qb+1); the
    # resulting early-qb ctx-MM stall is absorbed by the deep e-tile ring.
    ps = ctx.enter_context(tc.tile_pool(name="ps", bufs=3, space="PSUM"))
    es = ctx.enter_context(tc.tile_pool(name="es", bufs=12))
    cu = ctx.enter_context(tc.tile_pool(name="cu", bufs=2))
    sv = ctx.enter_context(tc.tile_pool(name="sv", bufs=2))
    bcp = ctx.enter_context(tc.tile_pool(name="bcp", bufs=2))
    csp = ctx.enter_context(tc.tile_pool(name="csp", bufs=2))
    osp = ctx.enter_context(tc.tile_pool(name="osp", bufs=2))
    rdp = ctx.enter_context(tc.tile_pool(name="rdp", bufs=2, space="DRAM"))

    def phase_c(ctxs, qsl):
        """Out-projection for one q-block.  o_ps tiles come from the ctxo
        ring, so this runs one q-block late: by then ctxs is long ready and
        the bc-chain latency is fully hidden."""
        for pr in range(2):
            o_ps = ps.tile([P, 1024], F32, tag="ctxo", bufs=1, name="o_ps")
            for i in range(2):
                e4 = pr * 2 + i
                mm(o_ps[:, i * 512:(i + 1) * 512],
                   wo_sb[:, e4 * P:(e4 + 1) * P], ctxs[:])
            o_sb = osp.tile([P, 1024], F32, tag="osb", name="o_sb")
            nc.vector.tensor_copy(o_sb[:], o_ps[:])
            for i in range(2):
                e4 = pr * 2 + i
                nc.sync.dma_start(out=yt[e4 * P:(e4 + 1) * P, qsl],
                                  in_=o_sb[:, i * 512:(i + 1) * 512])

    prev_c = None
    for qb in range(NQB):
        qsl = slice(qb * 512, (qb + 1) * 512)
        # ctx accumulator: h0 in cols 0:512, h1 in cols 512:1024; row 64 =
        # sums; rows 65:128 accumulate zeros (padded V weights)
        ctx2 = ps.tile([P, 1024], F32, tag="ctxo", bufs=1, name="ctx2")
        if prev_c is not None:
            phase_c(*prev_c)
        for k in range(NK):
            ksl = slice(k * P, (k + 1) * P)
            sp = ps.tile([P, 1024], F32, tag="s", name="sp")
            # two heads' scores: base_partition 0/64 slices -> row-tiled pair
            for h in range(2):
                hsl = slice(h * DH, (h + 1) * DH)
                mm(sp[:, h * 512:(h + 1) * 512],
                   kt_sb[hsl, ksl], qt_sb[hsl, qsl])
            e = es.tile([P, 1024], I16, tag="e", name="e")
            if _use_dve(k):
                # Schraudolph: int16 bits of fp16 e^(s/8)
                nc.vector.tensor_scalar(e[:], sp[:], SCHRAU_C, None,
                                        mybir.AluOpType.add)
            else:
                nc.scalar.activation(e[:].bitcast(F16), sp[:],
                                     mybir.ActivationFunctionType.Exp,
                                     scale=ACT_SCALE)
            ef = e[:].bitcast(F16)
            for h in range(2):
                vo = vones[h][:, k * VW:(k + 1) * VW]
                mm(ctx2[:, h * 512:(h + 1) * 512], vo,
                   ef[:, h * 512:(h + 1) * 512],
                   start=(k == 0), stop=(k == NK - 1))
        # drain ctx (unnormalized) to SBUF, freeing the psum bank
        ctxU = cu.tile([DH + 1, 1024], F32, tag="cu", name="ctxU")
        nc.vector.tensor_copy(ctxU[:], ctx2[0:DH + 1, :])
        # 1/sigma: DMA-reshape the sums row to [64, 16] (DVE reciprocal is
        # 8 cyc/elem along free dim -- spread it across partitions), exact
        # reciprocal, then broadcast via DRAM stride-0 bounce
        sg = sv.tile([DH, 16], F32, tag="sg", name="sg")
        nc.sync.dma_start(out=sg[:], in_=ctxU[DH:DH + 1, :])
        sr = sv.tile([DH, 16], F32, tag="sr", name="sr")
        nc.vector.reciprocal(sr[:], sg[:])
        sd = rdp.tile([1, 1024], F32, tag="sd", name="sd")
        nc.sync.dma_start(out=sd[0:1, :], in_=sr[:])
        bc = bcp.tile([DH, 1024], F32, tag="bc", name="bc")
        sd_bcast = bass.AP(tensor=sd.tensor, offset=sd.offset,
                           ap=[[0, DH]] + list(sd[0:1, :].ap)[1:])
        nc.sync.dma_start(out=bc[:], in_=sd_bcast)
        # normalized ctx, fp16, heads stacked on partitions for out-proj
        # (on GPSIMD: keeps ACT/DVE free for exp; SBUF-only operands)
        ctxs = csp.tile([P, 512], F16, tag="ctxs", name="ctxs")
        for h in range(2):
            nc.gpsimd.tensor_mul(ctxs[h * DH:(h + 1) * DH, :],
                                 ctxU[0:DH, h * 512:(h + 1) * 512],
                                 bc[:, h * 512:(h + 1) * 512])
        prev_c = (ctxs, qsl)
    phase_c(*prev_c)


_TPB_ENGINES = {mybir.EngineType.PE, mybir.EngineType.Activation,
                mybir.EngineType.DVE, mybir.EngineType.Pool}


def _legalize_matmul_waits(nc: bass.Bass) -> int:
    """Walrus encodes only ONE sync wait on TPB compute instructions (seen on
    Matmult and TensorCopy).  Move extra waits onto injected same-engine
    no-ops (one wait each) placed immediately before the instruction in its
    block: same semantics, legal encoding."""
    n_fixed = 0
    for f in nc.m.functions:
        for bb in f.blocks:
            out = []
            changed = False
            for ins in bb.instructions:
                si = ins.sync_info
                if (getattr(ins, "engine", None) is not None
                        and si is not None and len(si.on_wait) > 1):
                    for idx, w in enumerate(si.on_wait[:-1]):
                        nop = mybir.InstNoOp(name=f"{ins.name}-lgw{idx}",
                                             ins=[], outs=[])
                        nop.engine = ins.engine
                        nop.sync_info = mybir.SyncInfo(on_wait=[w], on_update=[])
                        out.append(nop)
                    ins.sync_info = mybir.SyncInfo(on_wait=[si.on_wait[-1]],
                                                   on_update=si.on_update)
                    n_fixed += 1
                    changed = True
                out.append(ins)
            if changed:
                bb.instructions = out
    return n_fixed


def build(S: int = S_FULL, legalize: bool = False) -> bass.Bass:
    nc = bass.Bass()
    with ExitStack() as ctx:
        ctx.enter_context(nc.allow_low_precision(
            reason="fp16 matmul operands / int16 exp bit-trick"))
        tc = ctx.enter_context(tile.TileContext(nc))
        _emit(nc, tc, ctx, S)
    if legalize:
        # only for the walrus/hardware path; CoreSim wants updates on every
        # instruction and doesn't enforce the 1-wait Matmult limit
        _legalize_matmul_waits(nc)
    return nc


_NC_CACHE = {}


def _get_nc(S: int) -> bass.Bass:
    if S not in _NC_CACHE:
        _NC_CACHE[S] = build(S, legalize=True)
    return _NC_CACHE[S]


def make_in_maps(X, Wq, Wk, Wv, Wo):
    X = np.asarray(X, np.float32)
    Wq = np.asarray(Wq, np.float32)
    Wk = np.asarray(Wk, np.float32)
    Wv = np.asarray(Wv, np.float32)
    Wo = np.asarray(Wo, np.float32)
    xts = [np.ascontiguousarray(X[b].T).astype(np.float16) for b in range(B)]
    in_maps = []
    for i in range(N_CORES):
        b, hp = divmod(i, 4)  # 4 head-pairs per batch
        csl = slice(hp * P, (hp + 1) * P)
        wqkv = np.concatenate([
            # fold the Schraudolph/ACT pre-scale into Wq
            Wq[:, csl] * LAM16, Wk[:, csl], Wv[:, csl]], axis=1)
        in_maps.append({
            "xt": xts[b],
            "wqkv": np.ascontiguousarray(wqkv).astype(np.float16),
            "wo": np.ascontiguousarray(Wo[csl, :]).astype(np.float16),
        })
    return in_maps


def kernel(X, Wq, Wk, Wv, Wo, _trace=False):
    global LAST_RESULTS
    X = np.asarray(X, dtype=np.float32)
    S = X.shape[1]
    nc = _get_nc(S)
    in_maps = make_in_maps(X, Wq, Wk, Wv, Wo)
    res = run_bass_kernel_spmd(nc, in_maps, list(range(N_CORES)), trace=_trace)
    LAST_RESULTS = res
    Y = np.zeros((B, S, D), dtype=np.float32)
    for i in range(N_CORES):
        Y[i // 4] += res.results[i]["yt"].T
    return Y


# revision 17
# speedup vs baseline: 1.0661x; 1.0504x over previous
"""Multi-head self-attention (B=2, S=4096, D=512, H=8, Dh=64) on 8 TRN2 cores.

Sharding: core i handles batch b = i//4 and head-pair hp = i%4 (heads 2*hp,
2*hp+1).  Each core computes Q/K/V projections for its two heads, flash-style
attention (no-max softmax; scores range is +-9 so exp is safe), and a partial
out-projection.  Host sums the 4 partial outputs per batch and transposes back.

v2 design (PE/ACT/DVE co-balanced, all-fp16 datapath):
  - All matmul operands fp16 (10-bit mantissa ~ f32r accuracy class).  fp16
    stationary operands get hidden LDWEIGHTS (pull-ahead) + fast weight load.
  - Scores for the two heads are emitted back-to-back with base_partition
    0/64 slices -> tile_position (0,0)/(64,0) row groups -> the PE runs them
    CONCURRENTLY (row tiling), halving scores stream time.
  - exp() alternates between ACT (exact, activation Exp) and DVE (Schraudolph
    int16 bit trick: e^(s/8) bits ~= trunc(1024*log2e*s/8 + C); the 1024*
    log2e/8 factor is folded into Wq host-side so scores arrive pre-scaled).
    One fused [128, 1024] tile covers both heads per k-tile.
  - V with a ones column appended per k-tile ([128, 65] weights); matmul with
    it accumulates context AND the softmax denominator (row 64) in one pass.
  - Normalize: ctx psum -> SBUF via DMA, reciprocal_approx_fast on the sums
    row, 1/sigma broadcast via DRAM stride-0 bounce, one DVE mul per head.
  - Out-projection per q-block; outputs DMA'd straight from PSUM to DRAM.

TRN2 quirk: walrus encodes only ONE sync wait on TPB compute instructions.
`_legalize_matmul_waits` moves extra waits onto injected single-wait no-ops.
"""

import sys
from contextlib import ExitStack

for _p in ("/opt/trn_rl_repo",):
    if _p not in sys.path:
        sys.path.insert(0, _p)

import numpy as np

import concourse.bass as bass
import concourse.tile as tile
from concourse import mybir
from concourse.bass_utils import run_bass_kernel_spmd

F32 = mybir.dt.float32
F16 = mybir.dt.float16
I16 = mybir.dt.int16
D = 512          # model dim
DH = 64          # head dim
P = 128          # partitions
B = 2
H = 8
S_FULL = 4096
N_CORES = 8
NC_T = D // P    # 4 contraction tiles over model dim

# Schraudolph fp16 exp: bits(e^(s/8)) ~= trunc(s_scaled + SCHRAU_C) where
# s_scaled = (1024*log2e/8) * s arrives pre-scaled (folded into Wq).
LAM16 = 1024.0 * np.log2(np.e) / 8.0          # 184.6644...
SCHRAU_C = 15315.75                            # tuned: max rel err +-3.0e-2
ACT_SCALE = float(np.log(2.0) / 1024.0)        # exp(ACT_SCALE * s_scaled)
# k-tiles whose exp runs on DVE (Schraudolph) instead of ACT (exact)
def _use_dve(k: int) -> bool:
    return (k % 8) in (2, 5, 7) or (k % 32) == 8   # 13/32 -> 40.6% on DVE

LAST_RESULTS = None  # test harness reads exec_time_ns from here


def _emit(nc: bass.Bass, tc: "tile.TileContext", ctx: ExitStack, S: int):
    NS = S // 512            # 512-wide seq blocks
    NK = S // P              # 128-row key tiles
    NQB = S // 512           # q blocks

    def mm(out, lhsT, rhs, start=True, stop=True):
        return nc.tensor.matmul(out, lhsT, rhs, start=start, stop=stop)

    xt = nc.declare_dram_parameter("xt", [D, S], F16, isOutput=False)
    wqkv = nc.declare_dram_parameter("wqkv", [D, 3 * P], F16, isOutput=False)
    wo = nc.declare_dram_parameter("wo", [P, D], F16, isOutput=False)
    yt = nc.declare_dram_parameter("yt", [D, S], F32, isOutput=True)

    const = ctx.enter_context(tc.tile_pool(name="const", bufs=1))

    # ---- weights to SBUF (one merged DMA per chunk + wo) ----
    wqkv_sb = []
    for c in range(NC_T):
        t = const.tile([P, 3 * P], F16, tag=f"wqkv{c}", name=f"wqkv{c}")
        nc.sync.dma_start(out=t[:], in_=wqkv[c * P:(c + 1) * P, :])
        wqkv_sb.append(t)
    w_sb = {name: [t[:, i * P:(i + 1) * P] for t in wqkv_sb]
            for i, name in enumerate(("wq", "wk", "wv"))}
    wo_sb = const.tile([P, D], F16, tag="wo")
    nc.sync.dma_start(out=wo_sb[:], in_=wo[:, :])

    # ---- xt to SBUF, one tile per (chunk, 1024-block), j-major order so
    # compute pipelines with the load; split across the two HWDGE queues ----
    NB = max(1, S // 1024)
    BW = min(S, 1024)
    xtc = [[None] * NB for _ in range(NC_T)]
    qeng = [nc.sync, nc.scalar]
    for j in range(NB):
        for c in range(NC_T):
            t = const.tile([P, BW], F16, tag=f"xt{c}_{j}", name=f"xt{c}_{j}")
            qeng[(j * NC_T + c) % 2].dma_start(
                out=t[:], in_=xt[c * P:(c + 1) * P, j * BW:(j + 1) * BW])
            xtc[c][j] = t

    def xts(c, j512):
        """[128, 512] view of xt chunk c, 512-block j512."""
        t = xtc[c][j512 // (BW // 512)]
        off = (j512 % (BW // 512)) * 512
        return t[:, off:off + 512]

    # persistent intermediates (all fp16)
    qt_sb = const.tile([P, S], F16, tag="qt")      # [2*64 d, S], pre-scaled
    kt_sb = const.tile([P, S], F16, tag="kt")
    # V padded to 128 cols per k-tile: [v(64) | ones(1) | zeros(63)].  The
    # 128-wide fp16 weight gets fast-weight-load; rows 65:128 of the ctx psum
    # accumulate zeros and are ignored.
    VW = P
    vones = [const.tile([P, NK * VW], F16, tag=f"vones{h}",
                        name=f"vones{h}") for h in range(2)]
    for h in range(2):
        vv = vones[h].rearrange("p (k c) -> p k c", c=VW)
        nc.vector.memset(vv[:, :, DH:], 0.0)
        nc.vector.memset(vv[:, :, DH:DH + 1], 1.0)

    # ---- phase A: projections ----
    with tc.tile_pool(name="pa", bufs=2, space="PSUM") as pa:
        for j in range(NS):
            jsl = slice(j * 512, (j + 1) * 512)
            pq = pa.tile([P, 512], F32, tag="pq", name="pq")
            for c in range(NC_T):
                mm(pq[:], w_sb["wq"][c], xts(c, j),
                   start=(c == 0), stop=(c == NC_T - 1))
            nc.vector.tensor_copy(qt_sb[:, jsl], pq[:])
            pk = pa.tile([P, 512], F32, tag="pk", name="pk")
            for c in range(NC_T):
                mm(pk[:], w_sb["wk"][c], xts(c, j),
                   start=(c == 0), stop=(c == NC_T - 1))
            nc.scalar.copy(kt_sb[:, jsl], pk[:])
            for t in range(4):
                k = j * 4 + t
                tsl = slice(t * P, (t + 1) * P)
                pv = pa.tile([P, P], F32, tag="pv", name="pv")
                for c in range(NC_T):
                    mm(pv[:], xts(c, j)[:, tsl], w_sb["wv"][c],
                       start=(c == 0), stop=(c == NC_T - 1))
                nc.vector.tensor_copy(
                    vones[0][:, k * VW:k * VW + DH], pv[:, 0:DH])
                nc.scalar.copy(
                    vones[1][:, k * VW:k * VW + DH], pv[:, DH:P])

    # ---- phase B: attention + phase C fused per q-block ----
    # PSUM budget: s 3x[128,1024]=6 banks + ctxo 1x[128,1024]=2 banks = 8.
    # The ctxo ring serves ctx2(qb) -> o0(qb) -> o1(qb) -> ctx2(qb+1); the
    # resulting early-qb ctx-MM stall is absorbed by the deep e-tile ring.
    ps = ctx.enter_context(tc.tile_pool(name="ps", bufs=3, space="PSUM"))
    es = ctx.enter_context(tc.tile_pool(name="es", bufs=12))
    cu = ctx.enter_context(tc.tile_pool(name="cu", bufs=2))
    sv = ctx.enter_context(tc.tile_pool(name="sv", bufs=2))
    bcp = ctx.enter_context(tc.tile_pool(name="bcp", bufs=2))
    csp = ctx.enter_context(tc.tile_pool(name="csp", bufs=3))
    osp = ctx.enter_context(tc.tile_pool(name="osp", bufs=2))
    rdp = ctx.enter_context(tc.tile_pool(name="rdp", bufs=2, space="DRAM"))

    def phase_c(ctxs, qsl):
        """Out-projection for one q-block.  o_ps tiles come from the ctxo
        ring and are emitted two q-blocks late: the o-matmuls end up gated
        only on the ring banks (previous ctx2 lifetime), with ctxs and the
        bc-chain latency long resolved."""
        for pr in range(2):
            o_ps = ps.tile([P, 1024], F32, tag="ctxo", bufs=1, name="o_ps")
            for i in range(2):
                e4 = pr * 2 + i
                mm(o_ps[:, i * 512:(i + 1) * 512],
                   wo_sb[:, e4 * P:(e4 + 1) * P], ctxs[:])
            o_sb = osp.tile([P, 1024], F32, tag="osb", name="o_sb")
            if pr == 0:
                nc.scalar.copy(o_sb[:], o_ps[:])
            else:
                nc.vector.tensor_copy(o_sb[:], o_ps[:])
            for i in range(2):
                e4 = pr * 2 + i
                nc.sync.dma_start(out=yt[e4 * P:(e4 + 1) * P, qsl],
                                  in_=o_sb[:, i * 512:(i + 1) * 512])

    pend_c = []
    for qb in range(NQB):
        qsl = slice(qb * 512, (qb + 1) * 512)
        # ctx accumulator: h0 in cols 0:512, h1 in cols 512:1024; row 64 =
        # sums; rows 65:128 accumulate zeros (padded V weights)
        ctx2 = ps.tile([P, 1024], F32, tag="ctxo", bufs=1, name="ctx2")
        if len(pend_c) >= 2:
            phase_c(*pend_c.pop(0))
        for k in range(NK):
            ksl = slice(k * P, (k + 1) * P)
            sp = ps.tile([P, 1024], F32, tag="s", name="sp")
            # two heads' scores: base_partition 0/64 slices -> row-tiled pair
            for h in range(2):
                hsl = slice(h * DH, (h + 1) * DH)
                mm(sp[:, h * 512:(h + 1) * 512],
                   kt_sb[hsl, ksl], qt_sb[hsl, qsl])
            e = es.tile([P, 1024], I16, tag="e", name="e")
            if _use_dve(k):
                # Schraudolph: int16 bits of fp16 e^(s/8)
                nc.vector.tensor_scalar(e[:], sp[:], SCHRAU_C, None,
                                        mybir.AluOpType.add)
            else:
                nc.scalar.activation(e[:].bitcast(F16), sp[:],
                                     mybir.ActivationFunctionType.Exp,
                                     scale=ACT_SCALE)
            ef = e[:].bitcast(F16)
            for h in range(2):
                vo = vones[h][:, k * VW:(k + 1) * VW]
                mm(ctx2[:, h * 512:(h + 1) * 512], vo,
                   ef[:, h * 512:(h + 1) * 512],
                   start=(k == 0), stop=(k == NK - 1))
        # drain ctx (unnormalized) to SBUF, freeing the psum bank
        ctxU = cu.tile([DH + 1, 1024], F32, tag="cu", name="ctxU")
        nc.vector.tensor_copy(ctxU[:], ctx2[0:DH + 1, :])
        # 1/sigma: DMA-reshape the sums row to [64, 16] (DVE reciprocal is
        # 8 cyc/elem along free dim -- spread it across partitions), exact
        # reciprocal, then broadcast via DRAM stride-0 bounce
        sg = sv.tile([DH, 16], F32, tag="sg", name="sg")
        nc.sync.dma_start(out=sg[:], in_=ctxU[DH:DH + 1, :])
        sr = sv.tile([DH, 16], F32, tag="sr", name="sr")
        nc.vector.reciprocal(sr[:], sg[:])
        sd = rdp.tile([1, 1024], F32, tag="sd", name="sd")
        nc.sync.dma_start(out=sd[0:1, :], in_=sr[:])
        bc = bcp.tile([DH, 1024], F32, tag="bc", name="bc")
        sd_bcast = bass.AP(tensor=sd.tensor, offset=sd.offset,
                           ap=[[0, DH]] + list(sd[0:1, :].ap)[1:])
        nc.sync.dma_start(out=bc[:], in_=sd_bcast)
        # normalized ctx, fp16, heads stacked on partitions for out-proj
        # (on GPSIMD: keeps ACT/DVE free for exp; SBUF-only operands)
        ctxs = csp.tile([P, 512], F16, tag="ctxs", name="ctxs")
        for h in range(2):
            nc.gpsimd.tensor_mul(ctxs[h * DH:(h + 1) * DH, :],
                                 ctxU[0:DH, h * 512:(h + 1) * 512],
                                 bc[:, h * 512:(h + 1) * 512])
        pend_c.append((ctxs, qsl))
    for pc in pend_c:
        phase_c(*pc)


_TPB_ENGINES = {mybir.EngineType.PE, mybir.EngineType.Activation,
                mybir.EngineType.DVE, mybir.EngineType.Pool}


def _legalize_matmul_waits(nc: bass.Bass) -> int:
    """Walrus encodes only ONE sync wait on TPB compute instructions (seen on
    Matmult and TensorCopy).  Move extra waits onto injected same-engine
    no-ops (one wait each) placed immediately before the instruction in its
    block: same semantics, legal encoding."""
    n_fixed = 0
    for f in nc.m.functions:
        for bb in f.blocks:
            out = []
            changed = False
            for ins in bb.instructions:
                si = ins.sync_info
                if (getattr(ins, "engine", None) is not None
                        and si is not None and len(si.on_wait) > 1):
                    for idx, w in enumerate(si.on_wait[:-1]):
                        nop = mybir.InstNoOp(name=f"{ins.name}-lgw{idx}",
                                             ins=[], outs=[])
                        nop.engine = ins.engine
                        nop.sync_info = mybir.SyncInfo(on_wait=[w], on_update=[])
                        out.append(nop)
                    ins.sync_info = mybir.SyncInfo(on_wait=[si.on_wait[-1]],
                                                   on_update=si.on_update)
                    n_fixed += 1
                    changed = True
                out.append(ins)
            if changed:
                bb.instructions = out
    return n_fixed


def build(S: int = S_FULL, legalize: bool = False) -> bass.Bass:
    nc = bass.Bass()
    with ExitStack() as ctx:
        ctx.enter_context(nc.allow_low_precision(
            reason="fp16 matmul operands / int16 exp bit-trick"))
        tc = ctx.enter_context(tile.TileContext(nc))
        _emit(nc, tc, ctx, S)
    if legalize:
        # only for the walrus/hardware path; CoreSim wants updates on every
        # instruction and doesn't enforce the 1-wait Matmult limit
        _legalize_matmul_waits(nc)
    return nc


_NC_CACHE = {}


def _get_nc(S: int) -> bass.Bass:
    if S not in _NC_CACHE:
        _NC_CACHE[S] = build(S, legalize=True)
    return _NC_CACHE[S]


def make_in_maps(X, Wq, Wk, Wv, Wo):
    X = np.asarray(X, np.float32)
    Wq = np.asarray(Wq, np.float32)
    Wk = np.asarray(Wk, np.float32)
    Wv = np.asarray(Wv, np.float32)
    Wo = np.asarray(Wo, np.float32)
    xts = [np.ascontiguousarray(X[b].T).astype(np.float16) for b in range(B)]
    in_maps = []
    for i in range(N_CORES):
        b, hp = divmod(i, 4)  # 4 head-pairs per batch
        csl = slice(hp * P, (hp + 1) * P)
        wqkv = np.concatenate([
            # fold the Schraudolph/ACT pre-scale into Wq
            Wq[:, csl] * LAM16, Wk[:, csl], Wv[:, csl]], axis=1)
        in_maps.append({
            "xt": xts[b],
            "wqkv": np.ascontiguousarray(wqkv).astype(np.float16),
            "wo": np.ascontiguousarray(Wo[csl, :]).astype(np.float16),
        })
    return in_maps


def kernel(X, Wq, Wk, Wv, Wo, _trace=False):
    global LAST_RESULTS
    X = np.asarray(X, dtype=np.float32)
    S = X.shape[1]
    nc = _get_nc(S)
    in_maps = make_in_maps(X, Wq, Wk, Wv, Wo)
    res = run_bass_kernel_spmd(nc, in_maps, list(range(N_CORES)), trace=_trace)
    LAST_RESULTS = res
    Y = np.zeros((B, S, D), dtype=np.float32)
    for i in range(N_CORES):
        Y[i // 4] += res.results[i]["yt"].T
    return Y


# revision 23
# speedup vs baseline: 1.1945x; 1.1204x over previous
"""Multi-head self-attention (B=2, S=4096, D=512, H=8, Dh=64) on 8 TRN2 cores.

Sharding: core i handles batch b = i//4 and head-pair hp = i%4 (heads 2*hp,
2*hp+1).  Each core computes Q/K/V projections for its two heads, flash-style
attention (no-max softmax; scores range is +-9 so exp is safe), and a partial
out-projection.  Host sums the 4 partial outputs per batch and transposes back.

v2 design (PE/ACT/DVE co-balanced, all-fp16 datapath):
  - All matmul operands fp16 (10-bit mantissa ~ f32r accuracy class).  fp16
    stationary operands get hidden LDWEIGHTS (pull-ahead) + fast weight load.
  - Scores for the two heads are emitted back-to-back with base_partition
    0/64 slices -> tile_position (0,0)/(64,0) row groups -> the PE runs them
    CONCURRENTLY (row tiling), halving scores stream time.
  - exp() alternates between ACT (exact, activation Exp) and DVE (Schraudolph
    int16 bit trick: e^(s/8) bits ~= trunc(1024*log2e*s/8 + C); the 1024*
    log2e/8 factor is folded into Wq host-side so scores arrive pre-scaled).
    One fused [128, 1024] tile covers both heads per k-tile.
  - V with a ones column appended per k-tile ([128, 65] weights); matmul with
    it accumulates context AND the softmax denominator (row 64) in one pass.
  - Normalize: ctx psum -> SBUF via DMA, reciprocal_approx_fast on the sums
    row, 1/sigma broadcast via DRAM stride-0 bounce, one DVE mul per head.
  - Out-projection per q-block; outputs DMA'd straight from PSUM to DRAM.

TRN2 quirk: walrus encodes only ONE sync wait on TPB compute instructions.
`_legalize_matmul_waits` moves extra waits onto injected single-wait no-ops.
"""

import sys
from contextlib import ExitStack

for _p in ("/opt/trn_rl_repo",):
    if _p not in sys.path:
        sys.path.insert(0, _p)

import numpy as np

import concourse.bass as bass
import concourse.tile as tile
from concourse import mybir
from concourse.bass_utils import run_bass_kernel_spmd

F32 = mybir.dt.float32
F16 = mybir.dt.float16
I16 = mybir.dt.int16
D = 512          # model dim
DH = 64          # head dim
P = 128          # partitions
B = 2
H = 8
S_FULL = 4096
N_CORES = 8
NC_T = D // P    # 4 contraction tiles over model dim

# Schraudolph fp16 exp: bits(e^(s/8)) ~= trunc(s_scaled + SCHRAU_C) where
# s_scaled = (1024*log2e/8) * s arrives pre-scaled (folded into Wq).
LAM16 = 1024.0 * np.log2(np.e) / 8.0          # 184.6644...
SCHRAU_C = 15315.75                            # tuned: max rel err +-3.0e-2
ACT_SCALE = float(np.log(2.0) / 1024.0)        # exp(ACT_SCALE * s_scaled)
# k-tiles whose exp runs on DVE (Schraudolph) instead of ACT (exact)
def _use_dve(k: int) -> bool:
    return (k % 8) in (2, 5, 7) or (k % 32) == 8   # 13/32 -> 40.6% on DVE

LAST_RESULTS = None  # test harness reads exec_time_ns from here


def _emit(nc: bass.Bass, tc: "tile.TileContext", ctx: ExitStack, S: int):
    NS = S // 512            # 512-wide seq blocks
    NK = S // P              # 128-row key tiles
    NQB = S // 512           # q blocks

    def mm(out, lhsT, rhs, start=True, stop=True):
        return nc.tensor.matmul(out, lhsT, rhs, start=start, stop=stop)

    xt = nc.declare_dram_parameter("xt", [D, S], F16, isOutput=False)
    wqkv = nc.declare_dram_parameter("wqkv", [D, 3 * P], F16, isOutput=False)
    wo = nc.declare_dram_parameter("wo", [P, D], F16, isOutput=False)
    yt = nc.declare_dram_parameter("yt", [D, S], F16, isOutput=True)

    const = ctx.enter_context(tc.tile_pool(name="const", bufs=1))

    # ---- weights to SBUF (one merged DMA per chunk + wo) ----
    wqkv_sb = []
    for c in range(NC_T):
        t = const.tile([P, 3 * P], F16, tag=f"wqkv{c}", name=f"wqkv{c}")
        nc.sync.dma_start(out=t[:], in_=wqkv[c * P:(c + 1) * P, :])
        wqkv_sb.append(t)
    w_sb = {name: [t[:, i * P:(i + 1) * P] for t in wqkv_sb]
            for i, name in enumerate(("wq", "wk", "wv"))}
    wo_sb = const.tile([P, D], F16, tag="wo")

    # ---- xt to SBUF, one tile per (chunk, 1024-block), j-major order so
    # compute pipelines with the load; split across the two HWDGE queues ----
    NB = max(1, S // 1024)
    BW = min(S, 1024)
    xtc = [[None] * NB for _ in range(NC_T)]
    qeng = [nc.sync, nc.scalar]
    for j in range(NB):
        for c in range(NC_T):
            t = const.tile([P, BW], F16, tag=f"xt{c}_{j}", name=f"xt{c}_{j}")
            qeng[(j * NC_T + c) % 2].dma_start(
                out=t[:], in_=xt[c * P:(c + 1) * P, j * BW:(j + 1) * BW])
            xtc[c][j] = t
    # wo is only needed by the (lagged) out-projection; load it last
    nc.scalar.dma_start(out=wo_sb[:], in_=wo[:, :])

    def xts(c, j512):
        """[128, 512] view of xt chunk c, 512-block j512."""
        t = xtc[c][j512 // (BW // 512)]
        off = (j512 % (BW // 512)) * 512
        return t[:, off:off + 512]

    # persistent intermediates (all fp16)
    qt_sb = const.tile([P, S], F16, tag="qt")      # [2*64 d, S], pre-scaled
    kt_sb = const.tile([P, S], F16, tag="kt")
    # V padded to 128 cols per k-tile: [v(64) | ones(1) | zeros(63)].  The
    # 128-wide fp16 weight gets fast-weight-load; rows 65:128 of the ctx psum
    # accumulate zeros and are ignored.
    VW = P
    vones = [const.tile([P, NK * VW], F16, tag=f"vones{h}",
                        name=f"vones{h}") for h in range(2)]
    for h in range(2):
        vv = vones[h].rearrange("p (k c) -> p k c", c=VW)
        nc.vector.memset(vv[:, :, DH:], 0.0)
        nc.vector.memset(vv[:, :, DH:DH + 1], 1.0)

    # ---- phase A: projections ----
    with tc.tile_pool(name="pa", bufs=2, space="PSUM") as pa:
        for j in range(NS):
            jsl = slice(j * 512, (j + 1) * 512)
            pq = pa.tile([P, 512], F32, tag="pq", name="pq")
            for c in range(NC_T):
                mm(pq[:], w_sb["wq"][c], xts(c, j),
                   start=(c == 0), stop=(c == NC_T - 1))
            nc.vector.tensor_copy(qt_sb[:, jsl], pq[:])
            pk = pa.tile([P, 512], F32, tag="pk", name="pk")
            for c in range(NC_T):
                mm(pk[:], w_sb["wk"][c], xts(c, j),
                   start=(c == 0), stop=(c == NC_T - 1))
            nc.scalar.copy(kt_sb[:, jsl], pk[:])
            for t in range(4):
                k = j * 4 + t
                tsl = slice(t * P, (t + 1) * P)
                pv = pa.tile([P, P], F32, tag="pv", name="pv")
                for c in range(NC_T):
                    mm(pv[:], xts(c, j)[:, tsl], w_sb["wv"][c],
                       start=(c == 0), stop=(c == NC_T - 1))
                nc.vector.tensor_copy(
                    vones[0][:, k * VW:k * VW + DH], pv[:, 0:DH])
                nc.scalar.copy(
                    vones[1][:, k * VW:k * VW + DH], pv[:, DH:P])

    # ---- phase B: attention + phase C fused per q-block ----
    # PSUM budget: s 3x[128,1024]=6 banks + ctxo 1x[128,1024]=2 banks = 8.
    # The ctxo ring serves ctx2(qb) -> o0(qb) -> o1(qb) -> ctx2(qb+1); the
    # resulting early-qb ctx-MM stall is absorbed by the deep e-tile ring.
    ps = ctx.enter_context(tc.tile_pool(name="ps", bufs=3, space="PSUM"))
    es = ctx.enter_context(tc.tile_pool(name="es", bufs=12))
    cu = ctx.enter_context(tc.tile_pool(name="cu", bufs=2))
    sv = ctx.enter_context(tc.tile_pool(name="sv", bufs=2))
    bcp = ctx.enter_context(tc.tile_pool(name="bcp", bufs=2))
    csp = ctx.enter_context(tc.tile_pool(name="csp", bufs=2))
    osp = ctx.enter_context(tc.tile_pool(name="osp", bufs=2))
    rdp = ctx.enter_context(tc.tile_pool(name="rdp", bufs=2, space="DRAM"))

    def phase_c(ctxs, qsl):
        """Out-projection for one q-block.  o_ps tiles come from the ctxo
        ring and are emitted two q-blocks late: the o-matmuls end up gated
        only on the ring banks (previous ctx2 lifetime), with ctxs and the
        bc-chain latency long resolved."""
        for pr in range(2):
            o_ps = ps.tile([P, 1024], F32, tag="ctxo", bufs=1, name="o_ps")
            for i in range(2):
                e4 = pr * 2 + i
                mm(o_ps[:, i * 512:(i + 1) * 512],
                   wo_sb[:, e4 * P:(e4 + 1) * P], ctxs[:])
            o_sb = osp.tile([P, 1024], F16, tag="osb", name="o_sb")
            if pr == 0:
                nc.scalar.copy(o_sb[:], o_ps[:])
            else:
                nc.vector.tensor_copy(o_sb[:], o_ps[:])
            for i in range(2):
                e4 = pr * 2 + i
                nc.sync.dma_start(out=yt[e4 * P:(e4 + 1) * P, qsl],
                                  in_=o_sb[:, i * 512:(i + 1) * 512])

    pend_c = []
    for qb in range(NQB):
        qsl = slice(qb * 512, (qb + 1) * 512)
        # ctx accumulator: h0 in cols 0:512, h1 in cols 512:1024; row 64 =
        # sums; rows 65:128 accumulate zeros (padded V weights)
        ctx2 = ps.tile([P, 1024], F32, tag="ctxo", bufs=1, name="ctx2")
        if len(pend_c) >= 2:
            phase_c(*pend_c.pop(0))
        for k in range(NK):
            ksl = slice(k * P, (k + 1) * P)
            sp = ps.tile([P, 1024], F32, tag="s", name="sp")
            # two heads' scores: base_partition 0/64 slices -> row-tiled pair
            for h in range(2):
                hsl = slice(h * DH, (h + 1) * DH)
                mm(sp[:, h * 512:(h + 1) * 512],
                   kt_sb[hsl, ksl], qt_sb[hsl, qsl])
            e = es.tile([P, 1024], I16, tag="e", name="e")
            if _use_dve(k):
                # Schraudolph: int16 bits of fp16 e^(s/8)
                nc.vector.tensor_scalar(e[:], sp[:], SCHRAU_C, None,
                                        mybir.AluOpType.add)
            else:
                nc.scalar.activation(e[:].bitcast(F16), sp[:],
                                     mybir.ActivationFunctionType.Exp,
                                     scale=ACT_SCALE)
            ef = e[:].bitcast(F16)
            for h in range(2):
                vo = vones[h][:, k * VW:(k + 1) * VW]
                mm(ctx2[:, h * 512:(h + 1) * 512], vo,
                   ef[:, h * 512:(h + 1) * 512],
                   start=(k == 0), stop=(k == NK - 1))
        # drain ctx (unnormalized) to SBUF, freeing the psum bank
        ctxU = cu.tile([DH + 1, 1024], F32, tag="cu", name="ctxU")
        nc.vector.tensor_copy(ctxU[:], ctx2[0:DH + 1, :])
        # 1/sigma: DMA-reshape the sums row to [64, 16] (DVE reciprocal is
        # 8 cyc/elem along free dim -- spread it across partitions), exact
        # reciprocal, then broadcast via DRAM stride-0 bounce
        sg = sv.tile([DH, 16], F32, tag="sg", name="sg")
        nc.scalar.dma_start(out=sg[:], in_=ctxU[DH:DH + 1, :])
        sr = sv.tile([DH, 16], F32, tag="sr", name="sr")
        nc.vector.reciprocal(sr[:], sg[:])
        sd = rdp.tile([1, 1024], F32, tag="sd", name="sd")
        nc.scalar.dma_start(out=sd[0:1, :], in_=sr[:])
        bc = bcp.tile([DH, 1024], F32, tag="bc", name="bc")
        sd_bcast = bass.AP(tensor=sd.tensor, offset=sd.offset,
                           ap=[[0, DH]] + list(sd[0:1, :].ap)[1:])
        nc.scalar.dma_start(out=bc[:], in_=sd_bcast)
        # normalized ctx, fp16, heads stacked on partitions for out-proj
        # (on GPSIMD: keeps ACT/DVE free for exp; SBUF-only operands)
        ctxs = csp.tile([P, 512], F16, tag="ctxs", name="ctxs")
        for h in range(2):
            nc.gpsimd.tensor_mul(ctxs[h * DH:(h + 1) * DH, :],
                                 ctxU[0:DH, h * 512:(h + 1) * 512],
                                 bc[:, h * 512:(h + 1) * 512])
        pend_c.append((ctxs, qsl))
    for pc in pend_c:
        phase_c(*pc)


_TPB_ENGINES = {mybir.EngineType.PE, mybir.EngineType.Activation,
                mybir.EngineType.DVE, mybir.EngineType.Pool}


def _legalize_matmul_waits(nc: bass.Bass) -> int:
    """Walrus encodes only ONE sync wait on TPB compute instructions (seen on
    Matmult and TensorCopy).  Move extra waits onto injected same-engine
    no-ops (one wait each) placed immediately before the instruction in its
    block: same semantics, legal encoding."""
    n_fixed = 0
    for f in nc.m.functions:
        for bb in f.blocks:
            out = []
            changed = False
            for ins in bb.instructions:
                si = ins.sync_info
                if (getattr(ins, "engine", None) is not None
                        and si is not None and len(si.on_wait) > 1):
                    for idx, w in enumerate(si.on_wait[:-1]):
                        nop = mybir.InstNoOp(name=f"{ins.name}-lgw{idx}",
                                             ins=[], outs=[])
                        nop.engine = ins.engine
                        nop.sync_info = mybir.SyncInfo(on_wait=[w], on_update=[])
                        out.append(nop)
                    ins.sync_info = mybir.SyncInfo(on_wait=[si.on_wait[-1]],
                                                   on_update=si.on_update)
                    n_fixed += 1
                    changed = True
                out.append(ins)
            if changed:
                bb.instructions = out
    return n_fixed


def build(S: int = S_FULL, legalize: bool = False) -> bass.Bass:
    nc = bass.Bass()
    with ExitStack() as ctx:
        ctx.enter_context(nc.allow_low_precision(
            reason="fp16 matmul operands / int16 exp bit-trick"))
        tc = ctx.enter_context(tile.TileContext(nc))
        _emit(nc, tc, ctx, S)
    if legalize:
        # only for the walrus/hardware path; CoreSim wants updates on every
        # instruction and doesn't enforce the 1-wait Matmult limit
        _legalize_matmul_waits(nc)
    return nc


_NC_CACHE = {}


def _get_nc(S: int) -> bass.Bass:
    if S not in _NC_CACHE:
        _NC_CACHE[S] = build(S, legalize=True)
    return _NC_CACHE[S]


def make_in_maps(X, Wq, Wk, Wv, Wo):
    X = np.asarray(X, np.float32)
    Wq = np.asarray(Wq, np.float32)
    Wk = np.asarray(Wk, np.float32)
    Wv = np.asarray(Wv, np.float32)
    Wo = np.asarray(Wo, np.float32)
    xts = [np.ascontiguousarray(X[b].T).astype(np.float16) for b in range(B)]
    in_maps = []
    for i in range(N_CORES):
        b, hp = divmod(i, 4)  # 4 head-pairs per batch
        csl = slice(hp * P, (hp + 1) * P)
        wqkv = np.concatenate([
            # fold the Schraudolph/ACT pre-scale into Wq
            Wq[:, csl] * LAM16, Wk[:, csl], Wv[:, csl]], axis=1)
        in_maps.append({
            "xt": xts[b],
            "wqkv": np.ascontiguousarray(wqkv).astype(np.float16),
            "wo": np.ascontiguousarray(Wo[csl, :]).astype(np.float16),
        })
    return in_maps


def kernel(X, Wq, Wk, Wv, Wo, _trace=False):
    global LAST_RESULTS
    X = np.asarray(X, dtype=np.float32)
    S = X.shape[1]
    nc = _get_nc(S)
    in_maps = make_in_maps(X, Wq, Wk, Wv, Wo)
    res = run_bass_kernel_spmd(nc, in_maps, list(range(N_CORES)), trace=_trace)
    LAST_RESULTS = res
    Y = np.zeros((B, S, D), dtype=np.float32)
    for i in range(N_CORES):
        Y[i // 4] += res.results[i]["yt"].T
    return Y
